# revision 18
# baseline (speedup 1.0000x reference)
"""BernoulliRBF retrieval kernel for 8 trn2 NeuronCores.

Math: for each query n, over each reference set (pos/neg):
    score[n,m] = 2 xs_n.ys_m - |ys_m|^2 - |xs_n|^2 - wb
    log_count[n] = LSE_m score[n,m]
Outputs: log_p_x = log_pos - logaddexp(log_pos, log_neg), log_weight_count.

Device layout (per core; cores 0-3 hold pos shards, 4-7 neg shards,
M-sharded 8192 refs/core): queries on the PSUM partition axis, refs on
the free axis.

    psum[q, m] = sum_k 2xs[k, q] * ys[k, m]   (fp16 matmuls, stationary
                  operand = the query tile, fp32 PSUM accum)
    u[q, m] = exp(psum - Cd)                  ACT from PSUM, constant bias
    partial[q] = sum_m u[q, m] * f[m]         DVE tensor_tensor_reduce,
                  f[m] = exp(-|ys_m|^2 - Cw_block) with refs SORTED by
                  |ys|^2 so each 2048-block's f spans ~e^-20..1 (no bf16
                  underflow); fp32 accumulate
    host: log_count = Cd + Cw + log(sum partials) - |xs|^2 - wb in f64.

The ACT exp stream is the wall (~127us busy).  The flipped layout makes
the startup need only ~320KB of input (one query tile + the first ref
columns) before the exp stream starts, the per-ref weight rides the DVE
reduce instead of the ACT bias, and the kernel output is a single 34KB
partials tile, so there is no store traffic competing with input DMA
and almost no tail.

Unit schedule (m-outer): pass 0 covers m[0:2048] for all 16 query
tiles, with the first tiles split finer so exp starts as soon as the
first 512 ref columns land; passes 1-3 cover the rest 2048 wide.  The
last unit is split in half so the final reduce+store overlaps the
second-to-last exp.
"""
import os
import numpy as np
from contextlib import ExitStack

N, M, D = 2048, 32768, 256
NCORES = 8
CORES_PER_SET = 4
SHARD = M // CORES_PER_SET      # 8192 refs per core
NQT = N // 128                  # 16 query tiles
CW_BLOCK = 2048                 # per-block shift granularity for f
SAMPLE_STRIDE = 16              # 512-point dot subsample for the Cd shift

# (q-tile, m0, m-width) unit schedule; one partials column per unit
UNITS = (
    [(0, 0, 512), (0, 512, 512), (0, 1024, 1024)]
    + [(1, 0, 1024), (1, 1024, 1024)]
    + [(qt, 0, 2048) for qt in range(2, NQT)]
    + [(qt, 2048, 2048) for qt in range(NQT)]
    + [(qt, 4096, 2048) for qt in range(NQT)]
    + [(qt, 6144, 2048) for qt in range(NQT - 1)]
    + [(NQT - 1, 6144, 1024), (NQT - 1, 7168, 1024)]
)
NUNITS = len(UNITS)             # 68

LAST_EXEC_NS = None             # set when BASS_TRACE=1

_cache = {}


def _build():
    import concourse.tile as tile
    from concourse import bacc, mybir

    F32, F16, BF16 = mybir.dt.float32, mybir.dt.float16, mybir.dt.bfloat16

    nc = bacc.Bacc("TRN2", target_bir_lowering=False, debug=False)
    # stationary operand: queries, [k-half, k, n]
    XS = nc.dram_tensor("XS", [2, 128, N], F16, kind="ExternalInput").ap()
    # moving operand: refs, [k-half, k, m]
    YS = nc.dram_tensor("YS", [2, 128, SHARD], F16, kind="ExternalInput").ap()
    # per-ref weight exp(w - Cw_block), replicated across partitions
    F = nc.dram_tensor("F", [128, SHARD], BF16, kind="ExternalInput").ap()
    # exp bias (-Cd), replicated [128, 1]
    BIAS = nc.dram_tensor("BIAS", [128, 1], F32, kind="ExternalInput").ap()
    # per-(query, unit) partial sums out
    P = nc.dram_tensor("P", [128, NUNITS], F32, kind="ExternalOutput").ap()

    with tile.TileContext(nc) as tc:
        with ExitStack() as ctx:
            sing = ctx.enter_context(tc.tile_pool(name="sing", bufs=1))
            psums = ctx.enter_context(tc.tile_pool(name="psum", bufs=2, space="PSUM"))
            upool = ctx.enter_context(tc.tile_pool(name="u", bufs=4))
            vpool = ctx.enter_context(tc.tile_pool(name="v", bufs=2))

            bias_sb = sing.tile([128, 1], F32)
            xs_sb = sing.tile([128, 2, N], F16)
            ys_sb = sing.tile([128, 2, SHARD], F16)
            f_sb = sing.tile([128, SHARD], BF16)
            part_sb = sing.tile([128, NUNITS], F32)

            # Startup wave on the two HWDGE queues, in consumption
            # order: bias, first ref columns, first query tile, then
            # ys/xs interleaved so pass 0 never starves.  The bulk
            # (later ys, f) rides gpsimd's SWDGE whose ~700ns/dma
            # descriptor-gen naturally staggers it off the critical
            # window.
            nc.sync.dma_start(out=bias_sb[:], in_=BIAS)
            nc.sync.dma_start(out=ys_sb[:, 0, 0:512], in_=YS[0][:, 0:512])
            nc.scalar.dma_start(out=ys_sb[:, 1, 0:512], in_=YS[1][:, 0:512])
            nc.sync.dma_start(out=xs_sb[:, 0, 0:256], in_=XS[0][:, 0:256])
            nc.scalar.dma_start(out=xs_sb[:, 1, 0:256], in_=XS[1][:, 0:256])
            nc.sync.dma_start(out=ys_sb[:, 0, 512:1024], in_=YS[0][:, 512:1024])
            nc.scalar.dma_start(out=ys_sb[:, 1, 512:1024], in_=YS[1][:, 512:1024])
            nc.sync.dma_start(out=ys_sb[:, 0, 1024:1536], in_=YS[0][:, 1024:1536])
            nc.scalar.dma_start(out=ys_sb[:, 1, 1024:1536], in_=YS[1][:, 1024:1536])
            nc.sync.dma_start(out=ys_sb[:, 0, 1536:2048], in_=YS[0][:, 1536:2048])
            nc.scalar.dma_start(out=ys_sb[:, 1, 1536:2048], in_=YS[1][:, 1536:2048])
            nc.sync.dma_start(out=xs_sb[:, 0, 256:1024], in_=XS[0][:, 256:1024])
            nc.scalar.dma_start(out=xs_sb[:, 1, 256:1024], in_=XS[1][:, 256:1024])
            nc.sync.dma_start(out=xs_sb[:, 0, 1024:2048], in_=XS[0][:, 1024:2048])
            nc.scalar.dma_start(out=xs_sb[:, 1, 1024:2048], in_=XS[1][:, 1024:2048])
            nc.gpsimd.dma_start(out=f_sb[:, 0:2048], in_=F[:, 0:2048])
            for c0 in range(2048, SHARD, 2048):
                sl = slice(c0, c0 + 2048)
                nc.gpsimd.dma_start(out=ys_sb[:, 0, sl], in_=YS[0][:, sl])
                nc.gpsimd.dma_start(out=ys_sb[:, 1, sl], in_=YS[1][:, sl])
                nc.gpsimd.dma_start(out=f_sb[:, sl], in_=F[:, sl])

            # PE warmup: opens the HAM clock gate while the startup
            # wave lands; no data dependency on any DMA.
            warm_w = sing.tile([128, 128], F16)
            warm_a = sing.tile([128, 256], F16)
            nc.vector.memset(warm_w[:], 0.0)
            nc.vector.memset(warm_a[:], 0.0)
            psum = psums.tile([128, 2048], F32)
            for _ in range(8):
                nc.tensor.matmul(
                    psum[:, 0:256], warm_w[:], warm_a[:], start=True, stop=True
                )

            for j, (qt, m0, mw) in enumerate(UNITS):
                psum = psums.tile([128, 2048], F32)
                for h in range(2):
                    lhsT = xs_sb[:, h, qt * 128:(qt + 1) * 128]
                    for c in range(0, mw, 512):
                        nc.tensor.matmul(
                            psum[:, c:c + 512],
                            lhsT,
                            ys_sb[:, h, m0 + c:m0 + c + 512],
                            start=(h == 0),
                            stop=(h == 1),
                        )
                u = upool.tile([128, 2048], BF16)
                nc.scalar.activation(
                    out=u[:, 0:mw],
                    in_=psum[:, 0:mw],
                    func=mybir.ActivationFunctionType.Exp,
                    bias=bias_sb[:, 0:1],
                    scale=1.0,
                )
                v = vpool.tile([128, 2048], BF16)
                nc.vector.affine_mul_reduce(
                    out=v[:, 0:mw],
                    accum_out=part_sb[:, j:j + 1],
                    in0=u[:, 0:mw],
                    in1=f_sb[:, m0:m0 + mw],
                    scale=1.0,
                    bias=0.0,
                )

            nc.sync.dma_start(out=P, in_=part_sb[:])

    nc.compile()
    return nc


def _prep_core(xs, ys_shard):
    """Host-side prep for one core's shard (refs already scale-applied).

    Sorts refs by |ys|^2 so the per-2048-block shifted f weights stay
    in bf16 normal range, builds the fp16 operands and the f/bias
    inputs, and returns the per-block shifts for the host combine.
    """
    w = -((ys_shard.astype(np.float64) ** 2).sum(axis=1))  # [SHARD]
    order = np.argsort(w)
    ys_s = ys_shard[order]
    w_s = w[order]
    YS_ = np.ascontiguousarray(ys_s.T).reshape(2, 128, SHARD).astype(np.float16)
    # per-2048-block shift for f
    cw = w_s.reshape(-1, CW_BLOCK).max(axis=1)             # [SHARD/CW_BLOCK]
    f = np.exp(w_s - np.repeat(cw, CW_BLOCK))
    F_ = np.ascontiguousarray(
        np.broadcast_to(f.astype(np.float32), (128, SHARD))
    )
    # dot-term shift from a subsample (a miss of a few only means u
    # values of e^few — harmless in bf16/fp32)
    t_s = 2.0 * (xs @ ys_s[::SAMPLE_STRIDE].T)
    Cd = float(t_s.max())
    bias = np.full((128, 1), -Cd, dtype=np.float32)
    return YS_, F_, bias, Cd, cw


def kernel(x, data_pos, data_neg, scales_pos, scales_neg, weight_bias):
    global LAST_EXEC_NS
    import ml_dtypes
    from concourse.bass_utils import run_bass_kernel_spmd

    x = np.asarray(x, dtype=np.float32)
    data_pos = np.asarray(data_pos, dtype=np.float32)
    data_neg = np.asarray(data_neg, dtype=np.float32)
    scales_pos = np.asarray(scales_pos, dtype=np.float32)
    scales_neg = np.asarray(scales_neg, dtype=np.float32)
    weight_bias = np.asarray(weight_bias, dtype=np.float32)

    if "nc" not in _cache:
        _cache["nc"] = _build()
    nc = _cache["nc"]

    in_maps = []
    meta = []
    XS_by_set = {}
    for core in range(NCORES):
        if core < CORES_PER_SET:
            key, data, scale = "p", data_pos, scales_pos
        else:
            key, data, scale = "n", data_neg, scales_neg
        xs = (x * scale[None, :]).astype(np.float32)
        ys = (data * scale[None, :]).astype(np.float32)
        sh = core % CORES_PER_SET
        ys_shard = ys[sh * SHARD:(sh + 1) * SHARD]
        YS_, F_, bias, Cd, cw = _prep_core(xs, ys_shard)
        if key not in XS_by_set:
            XS_ = np.ascontiguousarray((2.0 * xs).T).reshape(2, 128, N)
            XS_by_set[key] = XS_.astype(np.float16)
        in_maps.append(
            {
                "XS": XS_by_set[key],
                "YS": YS_,
                "F": F_.astype(ml_dtypes.bfloat16),
                "BIAS": bias,
            }
        )
        meta.append((Cd, cw))

    trace = os.environ.get("BASS_TRACE", "") not in ("", "0")
    try:
        res = run_bass_kernel_spmd(nc, in_maps, list(range(NCORES)), trace=trace)
    except ModuleNotFoundError:
        res = run_bass_kernel_spmd(nc, in_maps, list(range(NCORES)), trace=False)
    LAST_EXEC_NS = res.exec_time_ns

    # host combine in float64
    qt_of = np.array([u[0] for u in UNITS])
    blk_of = np.array([u[1] // CW_BLOCK for u in UNITS])

    def reduce_set(cores, xs64, wb):
        S = np.zeros(N)
        # common shift per set so the exp'd scales stay moderate
        C0 = max(meta[c][0] + meta[c][1].max() for c in cores)
        for core in cores:
            Cd, cw = meta[core]
            p = res.results[core]["P"].astype(np.float64)  # [128, NUNITS]
            scale_col = np.exp(Cd + cw[blk_of] - C0)       # [NUNITS]
            contrib = p * scale_col[None, :]
            for qt in range(NQT):
                cols = np.nonzero(qt_of == qt)[0]
                S[qt * 128:(qt + 1) * 128] += contrib[:, cols].sum(axis=1)
        return C0 + np.log(S) - (xs64 ** 2).sum(axis=1) - float(wb)

    xs_p = (x * scales_pos[None, :]).astype(np.float32).astype(np.float64)
    xs_n = (x * scales_neg[None, :]).astype(np.float32).astype(np.float64)
    log_pos = reduce_set(range(CORES_PER_SET), xs_p, weight_bias[0])
    log_neg = reduce_set(range(CORES_PER_SET, NCORES), xs_n, weight_bias[1])
    log_weight = np.logaddexp(log_pos, log_neg)
    log_p_x = log_pos - log_weight
    return (log_p_x.astype(np.float32), log_weight.astype(np.float32))


# revision 20
# speedup vs baseline: 1.3087x; 1.3087x over previous
"""BernoulliRBF retrieval kernel for 8 trn2 NeuronCores.

Math: for each query n, over each reference set (pos/neg):
    score[n,m] = 2 xs_n.ys_m - |ys_m|^2 - |xs_n|^2 - wb
    log_count[n] = LSE_m score[n,m]
Outputs: log_p_x = log_pos - logaddexp(log_pos, log_neg), log_weight_count.

Device layout (per core; cores 0-3 hold pos shards, 4-7 neg shards,
M-sharded 8192 refs/core), TRANSPOSED vs the usual: references live on
the partition axis, queries on the free axis.

    psum[p=ref, n=query] = sum_k ys[k, ref] * 2xs[k, query]   (fp16 matmuls,
                              1 cyc/col + fast weight load; fp32 PSUM accum)
    u = exp(psum + bias_p)    ACT straight from PSUM, bias = -|ys|^2 - C
                              per-partition (the whole reason for the
                              transposed layout), bf16 out
    DVE merges ref-tiles pairwise into tree roots (bf16 adds, 2x mode)
    roots DMA out; host reduces over partitions/roots/cores in float64.

The ACT exp stream is the wall (~126us busy).  Startup discipline: ONLY
the head-critical bytes (wb + all of a + the first 8 ref tiles, ~1.4MB,
split into many concurrent pieces) are kicked upfront so all 16 SDMA
engines work the critical path; the bulk ref chunks queue strictly
behind them on the same HWDGE queues.  Tiles 0-1 exp query sub-slices
as soon as their a-columns land; dummy matmuls keep the HAM clock gate
open until then.
"""
import os
import numpy as np
from contextlib import ExitStack

N, M, D = 2048, 32768, 256
NCORES = 8
CORES_PER_SET = 4
SHARD = M // CORES_PER_SET      # 8192 refs per core
NRT = SHARD // 128              # 64 ref tiles per core
TREE_SIZES = [8] * 7 + [4, 2]   # ref tiles per accumulation tree
RAW_TILES = 2                   # last tiles ship unmerged (short kernel tail)
NTREE = len(TREE_SIZES) + RAW_TILES
C_MARGIN = 12.0
SAMPLE_STRIDE = 64              # 512-point subsample for the C shift

LAST_EXEC_NS = None             # set when BASS_TRACE=1

_cache = {}


def _build():
    import concourse.tile as tile
    from concourse import bacc, mybir

    F32, F16, BF16 = mybir.dt.float32, mybir.dt.float16, mybir.dt.bfloat16

    nc = bacc.Bacc("TRN2", target_bir_lowering=False, debug=False)
    # moving operand: queries, [k-half, k, n]
    A = nc.dram_tensor("A", [2, 128, N], F16, kind="ExternalInput").ap()
    # stationary operand: refs, [k-half, k, m]
    B = nc.dram_tensor("B", [2, 128, SHARD], F16, kind="ExternalInput").ap()
    # per-ref bias (-|ys|^2 - C), [p, ref-tile]
    WB = nc.dram_tensor("WB", [128, NRT], F32, kind="ExternalInput").ap()
    # tree roots out
    U = nc.dram_tensor("U", [128, NTREE, N], BF16, kind="ExternalOutput").ap()

    with tile.TileContext(nc) as tc:
        with ExitStack() as ctx:
            sing = ctx.enter_context(tc.tile_pool(name="sing", bufs=1))
            psums = ctx.enter_context(tc.tile_pool(name="psum", bufs=2, space="PSUM"))
            upool = ctx.enter_context(tc.tile_pool(name="u", bufs=8))

            wb_sb = sing.tile([128, NRT], F32)
            a_sb = sing.tile([128, 2, N], F16)
            b_sb = sing.tile([128, 2, SHARD], F16)
            # Head wave: many concurrent pieces so the 16 SDMA engines
            # all pull critical bytes; a-columns in consumption order.
            nc.sync.dma_start(out=wb_sb[:], in_=WB)
            nc.sync.dma_start(out=b_sb[:, 0, 0:128], in_=B[0][:, 0:128])
            nc.scalar.dma_start(out=b_sb[:, 1, 0:128], in_=B[1][:, 0:128])
            for c in range(4):
                sl = slice(c * 512, (c + 1) * 512)
                nc.sync.dma_start(out=a_sb[:, 0, sl], in_=A[0][:, sl])
                nc.scalar.dma_start(out=a_sb[:, 1, sl], in_=A[1][:, sl])
                if c < 2:
                    bsl = slice(128 + c * 192, 128 + (c + 1) * 192)
                    nc.gpsimd.dma_start(out=b_sb[:, 0, bsl], in_=B[0][:, bsl])
                    nc.gpsimd.dma_start(out=b_sb[:, 1, bsl], in_=B[1][:, bsl])
            nc.gpsimd.dma_start(out=b_sb[:, 0, 512:1024], in_=B[0][:, 512:1024])
            nc.gpsimd.dma_start(out=b_sb[:, 1, 512:1024], in_=B[1][:, 512:1024])
            # Bulk ref chunks strictly behind the head on the HWDGE
            # queues: each queue is FIFO, so these only start draining
            # once the head pieces ahead of them are done.
            b_chunks = [1024, 2048, 3072, 4096, 5120, 6144, 7168, SHARD]
            for mc in range(len(b_chunks) - 1):
                sl = slice(b_chunks[mc], b_chunks[mc + 1])
                eng = nc.sync if mc % 2 == 0 else nc.scalar
                for h in range(2):
                    eng.dma_start(out=b_sb[:, h, sl], in_=B[h][:, sl])

            # PE warmup: dummy matmuls on zeroed tiles while the head
            # wave lands, so the HAM clock gate opens (1.2 -> 2.4 GHz)
            # before the first real tile.
            warm_w = sing.tile([128, 128], F16)
            warm_a = sing.tile([128, 512], F16)
            nc.vector.memset(warm_w[:], 0.0)
            nc.vector.memset(warm_a[:], 0.0)
            psum = psums.tile([128, N], F32)
            for _ in range(8):
                nc.tensor.matmul(
                    psum[:, 0:512], warm_w[:], warm_a[:], start=True, stop=True
                )

            def emit_tile(r):
                """MMs + exp for ref-tile r; returns its u tile (bf16).

                Ramp tiles (r<2) finish and exp query sub-slices as soon
                as their a-columns land, so the ACT chain starts early.
                """
                psum = psums.tile([128, N], F32)
                b_slices = [
                    b_sb[:, h, r * 128:(r + 1) * 128] for h in range(2)
                ]
                u = upool.tile([128, N], BF16)
                nq = 4 if r == 0 else (2 if r == 1 else 1)
                w = N // nq
                for piece in range(nq):
                    sl = slice(piece * w, (piece + 1) * w)
                    for h in range(2):
                        for c in range(piece * (w // 512), (piece + 1) * (w // 512)):
                            nc.tensor.matmul(
                                psum[:, c * 512:(c + 1) * 512],
                                b_slices[h],
                                a_sb[:, h, c * 512:(c + 1) * 512],
                                start=(h == 0),
                                stop=(h == 1),
                            )
                    nc.scalar.activation(
                        out=u[:, sl],
                        in_=psum[:, sl],
                        func=mybir.ActivationFunctionType.Exp,
                        bias=wb_sb[:, r:r + 1],
                        scale=1.0,
                    )
                return u

            r = 0
            for tr, tpt in enumerate(TREE_SIZES):
                # pairwise-merge tpt tiles (tpt in {8,4,2}) into one root
                stack = []      # (partial, level), pending pair merges
                for i in range(tpt):
                    u = emit_tile(r)
                    r += 1
                    node, lvl = u, 0
                    while stack and stack[-1][1] == lvl:
                        prev, _ = stack.pop()
                        merged = upool.tile([128, N], BF16)
                        nc.vector.tensor_add(merged[:], prev[:], node[:])
                        node, lvl = merged, lvl + 1
                    stack.append((node, lvl))
                root = stack[0][0]
                assert len(stack) == 1
                eng = nc.gpsimd if tr % 2 == 0 else nc.sync
                eng.dma_start(out=U[:, tr, :], in_=root[:])
            # tail tiles ship unmerged: nothing between the last exp and
            # its store; the final tile's exp is split in halves so its
            # first half streams out while the second is computed.
            for j in range(RAW_TILES):
                tr = len(TREE_SIZES) + j
                if j == RAW_TILES - 1:
                    psum = psums.tile([128, N], F32)
                    b_slices = [
                        b_sb[:, h, r * 128:(r + 1) * 128] for h in range(2)
                    ]
                    u = upool.tile([128, N], BF16)
                    for half in range(2):
                        hsl = slice(half * 1024, (half + 1) * 1024)
                        for h in range(2):
                            for c in (2 * half, 2 * half + 1):
                                nc.tensor.matmul(
                                    psum[:, c * 512:(c + 1) * 512],
                                    b_slices[h],
                                    a_sb[:, h, c * 512:(c + 1) * 512],
                                    start=(h == 0),
                                    stop=(h == 1),
                                )
                        nc.scalar.activation(
                            out=u[:, hsl],
                            in_=psum[:, hsl],
                            func=mybir.ActivationFunctionType.Exp,
                            bias=wb_sb[:, r:r + 1],
                            scale=1.0,
                        )
                        qa = slice(half * 1024, half * 1024 + 512)
                        qb = slice(half * 1024 + 512, (half + 1) * 1024)
                        nc.sync.dma_start(out=U[:, tr, qa], in_=u[:, qa])
                        nc.gpsimd.dma_start(out=U[:, tr, qb], in_=u[:, qb])
                else:
                    u = emit_tile(r)
                    nc.sync.dma_start(out=U[:, tr, 0:N // 2], in_=u[:, 0:N // 2])
                    nc.gpsimd.dma_start(out=U[:, tr, N // 2:], in_=u[:, N // 2:])
                r += 1
            assert r == NRT

    nc.compile()
    return nc


def _prep_set(x, data, scale):
    """Host-side prep for one reference set."""
    xs = (x * scale[None, :]).astype(np.float32)          # match reference rounding
    ys = (data * scale[None, :]).astype(np.float32)
    A = np.ascontiguousarray((2.0 * xs).T).reshape(2, 128, N).astype(np.float16)
    BT = np.ascontiguousarray(ys.T).reshape(2, 128, M).astype(np.float16)
    w = -((ys.astype(np.float64) ** 2).sum(axis=1))       # [M], float64
    # single per-set shift from a subsample
    samp = ys[::SAMPLE_STRIDE]
    t_s = 2.0 * (xs @ samp.T) + w[::SAMPLE_STRIDE][None, :].astype(np.float32)
    C = float(t_s.max()) + C_MARGIN
    xsq = (xs.astype(np.float64) ** 2).sum(axis=1)        # [N], float64
    return A, BT, w, C, xsq


def kernel(x, data_pos, data_neg, scales_pos, scales_neg, weight_bias):
    global LAST_EXEC_NS
    import ml_dtypes
    from concourse.bass_utils import run_bass_kernel_spmd

    x = np.asarray(x, dtype=np.float32)
    data_pos = np.asarray(data_pos, dtype=np.float32)
    data_neg = np.asarray(data_neg, dtype=np.float32)
    scales_pos = np.asarray(scales_pos, dtype=np.float32)
    scales_neg = np.asarray(scales_neg, dtype=np.float32)
    weight_bias = np.asarray(weight_bias, dtype=np.float32)

    if "nc" not in _cache:
        _cache["nc"] = _build()
    nc = _cache["nc"]

    prep_p = _prep_set(x, data_pos, scales_pos)
    prep_n = _prep_set(x, data_neg, scales_neg)

    in_maps = []
    for core in range(NCORES):
        A_, BT_, w_, C_, _ = prep_p if core < CORES_PER_SET else prep_n
        sh = core % CORES_PER_SET
        sl = slice(sh * SHARD, (sh + 1) * SHARD)
        wb = (w_[sl] - C_).astype(np.float32).reshape(NRT, 128).T
        in_maps.append(
            {
                "A": A_,
                "B": np.ascontiguousarray(BT_[:, :, sl]),
                "WB": np.ascontiguousarray(wb),
            }
        )

    trace = os.environ.get("BASS_TRACE", "") not in ("", "0")
    try:
        res = run_bass_kernel_spmd(nc, in_maps, list(range(NCORES)), trace=trace)
    except ModuleNotFoundError:
        # profiling hook unavailable in this environment — run untraced
        res = run_bass_kernel_spmd(nc, in_maps, list(range(NCORES)), trace=False)
    LAST_EXEC_NS = res.exec_time_ns

    # host combine in float64
    def reduce_set(cores, C, xsq, wb):
        tot = np.zeros(N)
        for core in cores:
            u = res.results[core]["U"]                    # [128, NTREE, N] bf16
            tot += u.astype(np.float64).sum(axis=(0, 1))
        return C + np.log(tot) - xsq - float(wb)

    log_pos = reduce_set(range(CORES_PER_SET), prep_p[3], prep_p[4], weight_bias[0])
    log_neg = reduce_set(
        range(CORES_PER_SET, NCORES), prep_n[3], prep_n[4], weight_bias[1]
    )
    log_weight = np.logaddexp(log_pos, log_neg)
    log_p_x = log_pos - log_weight
    return (log_p_x.astype(np.float32), log_weight.astype(np.float32))
